# revision 14
# baseline (speedup 1.0000x reference)
"""Trainium2 Bass kernel for a transformer block:
rms_norm -> GQA sliding-window attention (RoPE, tanh softcap) -> residual
-> rms_norm -> SwiGLU MLP -> residual, plus excess-kurtosis scalar.

Sharding: pure sequence-parallel over (batch=2) x (4 chunks of 512 tokens)
= 8 shards, one per NeuronCore.  Each core receives its token chunk plus a
512-token left halo (zero-padded for chunk 0), recomputes K/V for the halo
locally, and produces a disjoint 512-token slice of the output plus a
partial kurtosis sum.  No collectives; the host concatenates slices and
sums the 8 kurtosis partials.  Weights are replicated (bf16) - their DMA
hides under PE time since the block is compute-bound at bf16.

All matmuls run in bf16 (weights pre-cast on host); the residual stream,
softmax and normalizations are fp32.
"""
import os
os.environ.setdefault("JAX_COMPILATION_CACHE_DIR", "/tmp/jax_neff_cache")

import numpy as np
import ml_dtypes
from contextlib import ExitStack

import concourse.bass as bass
import concourse.mybir as mybir
import concourse.tile as tile
from concourse import bacc
from concourse.bass_utils import run_bass_kernel_spmd
from concourse.masks import make_identity

B, T, C = 2, 2048, 1024
NH, NKV, D = 16, 8, 64
R = NH // NKV
WINDOW = 512
HID = 4096
SOFT_CAP = 50.0
TQ = 512            # tokens per core
TEXT = 1024         # tokens incl. halo
NCHUNKS = T // TQ   # 4 chunks per batch

f32 = mybir.dt.float32
bf16 = mybir.dt.bfloat16
AF = mybir.ActivationFunctionType
ALU = mybir.AluOpType

_cache = {}


def _build_program():
    nc = bacc.Bacc("TRN2", target_bir_lowering=False, debug=False, num_devices=8)

    xext_d = nc.dram_tensor("xext", [TEXT, C], f32, kind="ExternalInput").ap()
    cos_d = nc.dram_tensor("cosd", [TEXT, 32], f32, kind="ExternalInput").ap()
    sin_d = nc.dram_tensor("sind", [TEXT, 32], f32, kind="ExternalInput").ap()
    mask_d = nc.dram_tensor("maskd", [8 * 128, 512], bf16, kind="ExternalInput").ap()
    s1_d = nc.dram_tensor("s1d", [1, C], f32, kind="ExternalInput").ap()
    s2_d = nc.dram_tensor("s2d", [1, C], f32, kind="ExternalInput").ap()
    qw_d = nc.dram_tensor("qwd", [C, NH * D], bf16, kind="ExternalInput").ap()
    kw_d = nc.dram_tensor("kwd", [C, NKV * D], bf16, kind="ExternalInput").ap()
    vw_d = nc.dram_tensor("vwd", [C, NKV * D], bf16, kind="ExternalInput").ap()
    ow_d = nc.dram_tensor("owd", [NH * D, C], bf16, kind="ExternalInput").ap()
    gw_d = nc.dram_tensor("gwd", [C, HID], bf16, kind="ExternalInput").ap()
    uw_d = nc.dram_tensor("uwd", [C, HID], bf16, kind="ExternalInput").ap()
    dw_d = nc.dram_tensor("dwd", [HID, C], bf16, kind="ExternalInput").ap()

    y_d = nc.dram_tensor("y", [TQ, C], f32, kind="ExternalOutput").ap()
    kurt_d = nc.dram_tensor("kurt", [1, 1], f32, kind="ExternalOutput").ap()

    with tile.TileContext(nc) as tc, ExitStack() as ctx:
        consts = ctx.enter_context(tc.tile_pool(name="consts", bufs=1))
        mid = ctx.enter_context(tc.tile_pool(name="mid", bufs=1))
        scr = ctx.enter_context(tc.tile_pool(name="scr", bufs=2))
        sml = ctx.enter_context(tc.tile_pool(name="sml", bufs=4))

        ident = consts.tile([128, 128], bf16)
        make_identity(nc, ident)
        ones128 = consts.tile([128, 1], f32)
        nc.vector.memset(ones128, 1.0)
        s1b = consts.tile([128, C], f32)
        nc.gpsimd.dma_start(out=s1b, in_=s1_d.to_broadcast([128, C]))
        s2b = consts.tile([128, C], f32)
        nc.gpsimd.dma_start(out=s2b, in_=s2_d.to_broadcast([128, C]))
        epsb = consts.tile([128, 1], f32)
        nc.vector.memset(epsb, 1e-6)
        kurt_cols = mid.tile([128, 4], f32)

        def rmsnorm(x_c, sb, out_bf):
            sq = scr.tile([128, C], f32, tag="scr4k", name="sq")
            ssq = sml.tile([128, 1], f32, tag="ssq", name="ssq")
            nc.scalar.activation(out=sq, in_=x_c, func=AF.Square, accum_out=ssq)
            sd = sml.tile([128, 1], f32, tag="sd", name="sd")
            nc.scalar.activation(out=sd, in_=ssq, func=AF.Sqrt, scale=1.0 / C,
                                 bias=epsb[:])
            rstd = sml.tile([128, 1], f32, tag="rstd", name="rstd")
            nc.vector.reciprocal(out=rstd, in_=sd)
            xnf = scr.tile([128, C], f32, tag="scr4kb", name="xnf")
            nc.vector.tensor_scalar_mul(xnf[:], x_c, rstd[:])
            nc.vector.tensor_mul(out_bf, xnf[:], sb[:])

        def transpose4(tpool, src_bf, dst, dst_chunk0, dst_cols):
            # transpose 4 adjacent [128,128] blocks of src into
            # dst[:, dst_chunk0:dst_chunk0+4, dst_cols]
            tp4 = tpool.tile([128, 512], bf16, tag="tp4", name="tp4")
            for b in range(4):
                nc.tensor.transpose(tp4[:, b * 128:(b + 1) * 128],
                                    src_bf[:, b * 128:(b + 1) * 128], ident[:])
            nc.vector.tensor_copy(
                out=dst[:, dst_chunk0:dst_chunk0 + 4, dst_cols],
                in_=tp4[:].rearrange("p (b n) -> p b n", b=4))

        with tc.tile_pool(name="attn_persist", bufs=1) as ap, \
             tc.tile_pool(name="xpool", bufs=2) as xp, \
             tc.tile_pool(name="ropes", bufs=2) as rp, \
             tc.tile_pool(name="smx", bufs=2) as smx:

            cos_t = ap.tile([128, 8, 32], f32)
            sin_t = ap.tile([128, 8, 32], f32)
            masks_t = ap.tile([128, 8, 512], bf16)
            nc.sync.dma_start(out=cos_t[:], in_=cos_d.rearrange("(rc p) f -> p rc f", p=128))
            nc.sync.dma_start(out=sin_t[:], in_=sin_d.rearrange("(rc p) f -> p rc f", p=128))
            nc.sync.dma_start(out=masks_t[:], in_=mask_d.rearrange("(rc p) f -> p rc f", p=128))
            qw_sb = ap.tile([128, 8, NH * D], bf16)
            kw_sb = ap.tile([128, 8, NKV * D], bf16)
            vw_sb = ap.tile([128, 8, NKV * D], bf16)
            ow_sb = ap.tile([128, 8, C], bf16)
            nc.sync.dma_start(out=qw_sb[:], in_=qw_d.rearrange("(kc p) n -> p kc n", p=128))
            nc.sync.dma_start(out=kw_sb[:], in_=kw_d.rearrange("(kc p) n -> p kc n", p=128))
            nc.sync.dma_start(out=vw_sb[:], in_=vw_d.rearrange("(kc p) n -> p kc n", p=128))
            nc.sync.dma_start(out=ow_sb[:], in_=ow_d.rearrange("(kc p) n -> p kc n", p=128))

            xq_t = ap.tile([128, 4, C], f32)       # query rows of x (residual)
            xnT_t = ap.tile([128, 8, TEXT], bf16)  # rms1(x) transposed [C, t]
            x2_t = mid.tile([128, 4, C], f32)      # x + attn_out
            KT_t = ap.tile([128, 4, TEXT], bf16)
            V_aug = ap.tile([128, 8, 8, 65], bf16)
            QT_t = ap.tile([128, 8, TQ], bf16)
            OT_pack = ap.tile([128, 8, TQ], bf16)

            def rope(nat_psum_view, rc, out_bf, nheads):
                # nat_psum_view: [128, nheads, 64] psum fp32; out_bf alike bf16
                cos_b = cos_t[:, rc:rc + 1, :].broadcast_to([128, nheads, 32])
                sin_b = sin_t[:, rc:rc + 1, :].broadcast_to([128, nheads, 32])
                lo, hi = nat_psum_view[:, :, 0:32], nat_psum_view[:, :, 32:64]
                t1 = rp.tile([128, nheads, 32], f32, tag="t1", name="t1")
                t2 = rp.tile([128, nheads, 32], f32, tag="t2", name="t2")
                nc.vector.tensor_mul(t1[:], lo, cos_b)
                nc.vector.tensor_mul(t2[:], hi, sin_b)
                nc.vector.tensor_sub(out_bf[:, :, 0:32], t1[:], t2[:])
                t3 = rp.tile([128, nheads, 32], f32, tag="t1", name="t3")
                t4 = rp.tile([128, nheads, 32], f32, tag="t2", name="t4")
                nc.vector.tensor_mul(t3[:], hi, cos_b)
                nc.vector.tensor_mul(t4[:], lo, sin_b)
                nc.vector.tensor_add(out_bf[:, :, 32:64], t3[:], t4[:])

            # ================= phases 1-2: norms + projections ============
            with tc.tile_pool(name="mmA", bufs=3, space="PSUM") as mmA, \
                 tc.tile_pool(name="tppA", bufs=2, space="PSUM") as tppA:

                nc.sync.dma_start(out=xq_t[:],
                                  in_=xext_d[512:].rearrange("(c p) n -> p c n", p=128))
                nc.gpsimd.memset(V_aug[:, :, :, 64:65], 1.0)
                for rc in range(8):
                    if rc >= 4:
                        x_c = xq_t[:, rc - 4, :]
                    else:
                        x_c = xp.tile([128, C], f32, tag="xc", name="xc")
                        nc.sync.dma_start(out=x_c, in_=xext_d[rc * 128:(rc + 1) * 128, :])
                    xnb = scr.tile([128, C], bf16, tag="xnb", name="xnb")
                    rmsnorm(x_c, s1b, xnb[:])
                    for half in range(2):
                        transpose4(tppA, xnb[:, half * 512:(half + 1) * 512],
                                   xnT_t, half * 4, slice(rc * 128, (rc + 1) * 128))

                for rc in range(8):
                    kps = mmA.tile([128, 512], f32, tag="mm", name="kps")
                    for kc in range(8):
                        nc.tensor.matmul(kps[:], xnT_t[:, kc, rc * 128:(rc + 1) * 128],
                                         kw_sb[:, kc, :], start=kc == 0, stop=kc == 7)
                    krb = rp.tile([128, 8, 64], bf16, tag="krb", name="krb")
                    rope(kps[:].rearrange("p (h d) -> p h d", h=8), rc, krb, 8)
                    transpose4(tppA, krb[:].rearrange("p h d -> p (h d)"),
                               KT_t, 0, slice(rc * 128, (rc + 1) * 128))

                    vps = mmA.tile([128, 512], f32, tag="mm", name="vps")
                    for kc in range(8):
                        nc.tensor.matmul(vps[:], xnT_t[:, kc, rc * 128:(rc + 1) * 128],
                                         vw_sb[:, kc, :], start=kc == 0, stop=kc == 7)
                    nc.vector.tensor_copy(out=V_aug[:, rc, :, 0:64],
                                          in_=vps[:].rearrange("p (h d) -> p h d", h=8))

                for icq in range(4):
                    rc = 4 + icq
                    for nh in range(2):
                        qps = mmA.tile([128, 512], f32, tag="mm", name="qps")
                        for kc in range(8):
                            nc.tensor.matmul(qps[:], xnT_t[:, kc, rc * 128:(rc + 1) * 128],
                                             qw_sb[:, kc, nh * 512:(nh + 1) * 512],
                                             start=kc == 0, stop=kc == 7)
                        qrb = rp.tile([128, 8, 64], bf16, tag="krb", name="qrb")
                        rope(qps[:].rearrange("p (h d) -> p h d", h=8), rc, qrb, 8)
                        transpose4(tppA, qrb[:].rearrange("p h d -> p (h d)"),
                                   QT_t, nh * 4, slice(icq * 128, (icq + 1) * 128))

            # ================= phases 3-4: attention + out-proj ===========
            with tc.tile_pool(name="spp", bufs=2, space="PSUM") as spp, \
                 tc.tile_pool(name="otp", bufs=2, space="PSUM") as otp, \
                 tc.tile_pool(name="mmB", bufs=2, space="PSUM") as mmB, \
                 tc.tile_pool(name="dscr", bufs=2, space="DRAM") as dp:

                for rh in range(NH):
                    kv = rh % NKV
                    ko = 64 * (kv % 2)
                    qo = 64 * (rh % 2)
                    ots = otp.tile([65, 512], f32, tag="ot", name="ots")
                    for jp in range(4):
                        sp_t = spp.tile([128, 1024], f32, tag="sp", name="sp_t")
                        for h2 in range(2):
                            jt = jp * 2 + h2
                            nc.tensor.matmul(
                                sp_t[:, h2 * 512:(h2 + 1) * 512],
                                KT_t[ko:ko + 64, kv // 2, jt * 128:(jt + 1) * 128],
                                QT_t[qo:qo + 64, rh // 2, :], start=True, stop=True)
                        tanh_t = smx.tile([128, 1024], f32, tag="tanh", name="tanh_t")
                        nc.scalar.activation(out=tanh_t[:], in_=sp_t[:], func=AF.Tanh)
                        p_bf = smx.tile([128, 1024], bf16, tag="pbf", name="p_bf")
                        nc.scalar.activation(out=p_bf[:], in_=tanh_t[:], func=AF.Exp,
                                             scale=SOFT_CAP)
                        nc.vector.tensor_mul(
                            p_bf[:], p_bf[:],
                            masks_t[:, jp * 2:(jp + 1) * 2, :].rearrange("p a b -> p (a b)"))
                        for h2 in range(2):
                            jt = jp * 2 + h2
                            nc.tensor.matmul(ots[:], V_aug[:, jt, kv, :],
                                             p_bf[:, h2 * 512:(h2 + 1) * 512],
                                             start=jt == 0, stop=jt == 7)
                    dinv = sml.tile([1, 512], f32, tag="dinv", name="dinv")
                    nc.vector.reciprocal(out=dinv, in_=ots[64:65, :])
                    dd = dp.tile([1, 512], f32, tag="dd", name="dd")
                    nc.sync.dma_start(out=dd[:], in_=dinv[:])
                    bcast = smx.tile([64, 512], f32, tag="bcast", name="bcast")
                    nc.sync.dma_start(out=bcast[:], in_=dd[:].to_broadcast([64, 512]))
                    nc.vector.tensor_mul(OT_pack[qo:qo + 64, rh // 2, :],
                                         ots[0:64, :], bcast[:])

                for mc in range(4):
                    for nh in range(2):
                        ops_ = mmB.tile([128, 512], f32, tag="mm", name="ops_")
                        for kc in range(8):
                            nc.tensor.matmul(ops_[:], OT_pack[:, kc, mc * 128:(mc + 1) * 128],
                                             ow_sb[:, kc, nh * 512:(nh + 1) * 512],
                                             start=kc == 0, stop=kc == 7)
                        nc.vector.tensor_add(x2_t[:, mc, nh * 512:(nh + 1) * 512], ops_[:],
                                             xq_t[:, mc, nh * 512:(nh + 1) * 512])

        # ================= phases 5-7: MLP + output ======================
        with tc.tile_pool(name="mlp_persist", bufs=1) as mp, \
             tc.tile_pool(name="wstream", bufs=2) as ws, \
             tc.tile_pool(name="ypool", bufs=1) as yp, \
             tc.tile_pool(name="kstat", bufs=2) as kp, \
             tc.tile_pool(name="mmC", bufs=3, space="PSUM") as mmC, \
             tc.tile_pool(name="tppC", bufs=2, space="PSUM") as tppC:

            xn2T_t = mp.tile([128, 8, TQ], bf16)
            for mc in range(4):
                xnb = scr.tile([128, C], bf16, tag="xnb", name="xnb2")
                rmsnorm(x2_t[:, mc, :], s2b, xnb[:])
                for half in range(2):
                    transpose4(tppC, xnb[:, half * 512:(half + 1) * 512],
                               xn2T_t, half * 4, slice(mc * 128, (mc + 1) * 128))

            hT_t = mp.tile([128, 32, TQ], bf16)
            for nh in range(8):
                gcol = ws.tile([128, 8, 512], bf16, tag="gcol")
                ucol = ws.tile([128, 8, 512], bf16, tag="ucol")
                nc.sync.dma_start(
                    out=gcol[:],
                    in_=gw_d[:, nh * 512:(nh + 1) * 512].rearrange("(kc p) n -> p kc n", p=128))
                nc.sync.dma_start(
                    out=ucol[:],
                    in_=uw_d[:, nh * 512:(nh + 1) * 512].rearrange("(kc p) n -> p kc n", p=128))
                for bp in range(2):
                    gps = mmC.tile([128, 1024], f32, tag="mm", name="gps")
                    ups = mmC.tile([128, 1024], f32, tag="mm", name="ups")
                    for h2 in range(2):
                        b2 = bp * 2 + h2
                        for kc in range(8):
                            nc.tensor.matmul(gps[:, h2 * 512:(h2 + 1) * 512],
                                             gcol[:, kc, b2 * 128:(b2 + 1) * 128],
                                             xn2T_t[:, kc, :], start=kc == 0, stop=kc == 7)
                        for kc in range(8):
                            nc.tensor.matmul(ups[:, h2 * 512:(h2 + 1) * 512],
                                             ucol[:, kc, b2 * 128:(b2 + 1) * 128],
                                             xn2T_t[:, kc, :], start=kc == 0, stop=kc == 7)
                    gs = ws.tile([128, 1024], f32, tag="gs", name="gs")
                    nc.scalar.activation(out=gs[:], in_=gps[:], func=AF.Sigmoid)
                    gf = ws.tile([128, 1024], f32, tag="hbf", name="gf")
                    nc.vector.tensor_mul(gf[:], gs[:], gps[:])
                    nc.vector.tensor_mul(
                        hT_t[:, nh * 4 + bp * 2:nh * 4 + bp * 2 + 2, :]
                        .rearrange("p a b -> p (a b)"), gf[:], ups[:])

            y_ts = [yp.tile([128, C], f32, tag=f"yc{mc}", name=f"yc{mc}")
                    for mc in range(4)]
            for nh2 in range(2):
                dwh = ws.tile([128, 32, 512], bf16, tag="dwh", bufs=1, name="dwh")
                nc.sync.dma_start(
                    out=dwh[:],
                    in_=dw_d[:, nh2 * 512:(nh2 + 1) * 512].rearrange("(kc p) n -> p kc n", p=128))
                for mc in range(4):
                    dps = mmC.tile([128, 512], f32, tag="mm", name="dps")
                    for kc in range(32):
                        nc.tensor.matmul(dps[:], hT_t[:, kc, mc * 128:(mc + 1) * 128],
                                         dwh[:, kc, :], start=kc == 0, stop=kc == 31)
                    nc.vector.tensor_add(y_ts[mc][:, nh2 * 512:(nh2 + 1) * 512], dps[:],
                                         x2_t[:, mc, nh2 * 512:(nh2 + 1) * 512])

            for mc in range(4):
                y_c = y_ts[mc]
                nc.sync.dma_start(out=y_d[mc * 128:(mc + 1) * 128, :], in_=y_c[:])
                stats = kp.tile([128, 2, 6], f32, tag="stats", name="stats")
                for sg in range(2):
                    nc.vector.bn_stats(out=stats[:, sg, :], in_=y_c[:, sg * 512:(sg + 1) * 512])
                mv = kp.tile([128, 2], f32, tag="mv", name="mv")
                nc.vector.bn_aggr(out=mv[:], in_=stats[:])
                c_t = scr.tile([128, C], f32, tag="scr4k", name="c_t")
                nc.vector.tensor_scalar_sub(c_t[:], y_c[:], mv[:, 0:1])
                c2 = scr.tile([128, C], f32, tag="scr4kb", name="c2")
                nc.scalar.activation(out=c2[:], in_=c_t[:], func=AF.Square)
                c4sc = scr.tile([128, C], f32, tag="scr4k", name="c4sc")
                c4s = sml.tile([128, 1], f32, tag="c4s", name="c4s")
                nc.scalar.activation(out=c4sc[:], in_=c2[:], func=AF.Square, accum_out=c4s)
                v2 = sml.tile([128, 1], f32, tag="v2", name="v2")
                nc.vector.tensor_mul(v2[:], mv[:, 1:2], mv[:, 1:2])
                v2b = sml.tile([128, 1], f32, tag="v2b", name="v2b")
                nc.vector.tensor_scalar_add(v2b[:], v2[:], 1e-6)
                v2i = sml.tile([128, 1], f32, tag="v2i", name="v2i")
                nc.vector.reciprocal(out=v2i, in_=v2b[:])
                kr = sml.tile([128, 1], f32, tag="kr", name="kr")
                nc.vector.tensor_mul(kr[:], c4s[:], v2i[:])
                kr2 = sml.tile([128, 1], f32, tag="kr2", name="kr2")
                nc.vector.tensor_scalar(out=kr2[:], in0=kr[:], scalar1=1.0 / C,
                                        scalar2=-3.0, op0=ALU.mult, op1=ALU.add)
                nc.vector.tensor_scalar_max(kurt_cols[:, mc:mc + 1], kr2[:], 0.0)

            krow = sml.tile([128, 1], f32, tag="krow", name="krow")
            nc.vector.tensor_reduce(out=krow[:], in_=kurt_cols[:],
                                    axis=mybir.AxisListType.X, op=ALU.add)
            kps_ = mmC.tile([64, 512], f32, tag="mm", name="kps_")
            nc.tensor.matmul(kps_[0:1, 0:1], ones128[:], krow[:], start=True, stop=True)
            kurt_sc = sml.tile([1, 1], f32, tag="ksc", name="ksc")
            nc.vector.tensor_copy(out=kurt_sc[:], in_=kps_[0:1, 0:1])
            nc.sync.dma_start(out=kurt_d, in_=kurt_sc[:])

    nc.compile()
    return nc


def _host_inputs(inputs):
    """Returns list of 8 per-core input maps."""
    x = np.asarray(inputs["x"], np.float32)
    rms1 = np.asarray(inputs["rms1_scale"], np.float32)
    rms2 = np.asarray(inputs["rms2_scale"], np.float32)

    def tobf(a):
        return np.ascontiguousarray(a.astype(ml_dtypes.bfloat16))

    qw = tobf(np.asarray(inputs["q_kernel"], np.float32).reshape(C, NH * D) * (0.125 / SOFT_CAP))
    kw = tobf(np.asarray(inputs["k_kernel"], np.float32).reshape(C, NKV * D))
    vw = tobf(np.asarray(inputs["v_kernel"], np.float32).reshape(C, NKV * D))
    ow = tobf(np.asarray(inputs["out_kernel"], np.float32).reshape(NH * D, C))
    gw = tobf(np.asarray(inputs["gate_kernel"], np.float32))
    uw = tobf(np.asarray(inputs["up_kernel"], np.float32))
    dw = tobf(np.asarray(inputs["down_kernel"], np.float32))
    s1 = np.ascontiguousarray((1.0 + rms1).reshape(1, C))
    s2 = np.ascontiguousarray((1.0 + rms2).reshape(1, C))

    # rope tables (fp32, replicating reference formula)
    fraction = (np.arange(0, D, 2, dtype=np.float32) / np.float32(D)).astype(np.float32)
    rot_freq = (np.float32(1.0) / np.power(np.float32(1e6), fraction)).astype(np.float32)
    pos = np.arange(T, dtype=np.float32)
    sinu = pos[:, None] * rot_freq[None, :]
    sin_full = np.sin(sinu).astype(np.float32)
    cos_full = np.cos(sinu).astype(np.float32)

    # band mask: valid iff il < jl <= il + WINDOW (local coords)
    jl = np.arange(TEXT)[:, None]
    il = np.arange(TQ)[None, :]
    band = (il < jl) & (jl <= il + WINDOW)

    in_maps = []
    for core in range(8):
        b, s = core // NCHUNKS, core % NCHUNKS
        i0 = s * TQ
        lo = i0 - WINDOW
        x_ext = np.zeros((TEXT, C), np.float32)
        cos_e = np.zeros((TEXT, 32), np.float32)
        sin_e = np.zeros((TEXT, 32), np.float32)
        src_lo = max(lo, 0)
        dst_lo = src_lo - lo
        x_ext[dst_lo:] = x[b, src_lo:i0 + TQ]
        cos_e[dst_lo:] = cos_full[src_lo:i0 + TQ]
        sin_e[dst_lo:] = sin_full[src_lo:i0 + TQ]
        valid = (lo + jl >= 0)
        mask_core = np.ascontiguousarray(
            (band & valid).reshape(8 * 128, 512).astype(ml_dtypes.bfloat16))
        in_maps.append({
            "xext": x_ext, "cosd": cos_e, "sind": sin_e, "maskd": mask_core,
            "s1d": s1, "s2d": s2,
            "qwd": qw, "kwd": kw, "vwd": vw, "owd": ow,
            "gwd": gw, "uwd": uw, "dwd": dw,
        })
    return in_maps


def kernel(**inputs):
    if "nc" not in _cache:
        _cache["nc"] = _build_program()
    nc = _cache["nc"]
    in_maps = _host_inputs(inputs)
    trace = bool(int(os.environ.get("TRNK_TRACE", "0")))
    res = run_bass_kernel_spmd(nc, in_maps, core_ids=list(range(8)), trace=trace)
    _cache["last_result"] = res

    x_out = np.empty((B, T, C), np.float32)
    kurt = np.float32(np.asarray(inputs["kurtosis_sum"], np.float32))
    for core in range(8):
        b, s = core // NCHUNKS, core % NCHUNKS
        x_out[b, s * TQ:(s + 1) * TQ] = res.results[core]["y"]
        kurt = np.float32(kurt + res.results[core]["kurt"][0, 0])
    return (x_out, kurt)


# revision 22
# speedup vs baseline: 105.0980x; 105.0980x over previous
"""Trainium2 Bass kernel for a transformer block:
rms_norm -> GQA sliding-window attention (RoPE, tanh softcap) -> residual
-> rms_norm -> SwiGLU MLP -> residual, plus excess-kurtosis scalar.

Sharding: pure sequence-parallel over (batch=2) x (4 chunks of 512 tokens)
= 8 shards, one per NeuronCore.  Each core receives its token chunk plus a
512-token left halo (zero-padded for chunk 0), recomputes K/V for the halo
locally, and produces a disjoint 512-token slice of the output plus a
partial kurtosis sum.  No collectives; the host concatenates slices and
sums the 8 kurtosis partials.  Weights are replicated (bf16) - their DMA
hides under PE time since the block is compute-bound at bf16.

All matmuls run in bf16 (weights pre-cast on host); the residual stream,
softmax and normalizations are fp32.
"""
import os
os.environ.setdefault("JAX_COMPILATION_CACHE_DIR", "/tmp/jax_neff_cache")

import numpy as np
import ml_dtypes
from contextlib import ExitStack

import concourse.bass as bass
import concourse.mybir as mybir
import concourse.tile as tile
from concourse import bacc
from concourse.bass_utils import run_bass_kernel_spmd
from concourse.masks import make_identity

B, T, C = 2, 2048, 1024
NH, NKV, D = 16, 8, 64
R = NH // NKV
WINDOW = 512
HID = 4096
SOFT_CAP = 50.0
TQ = 512            # tokens per core
TEXT = 1024         # tokens incl. halo
NCHUNKS = T // TQ   # 4 chunks per batch

f32 = mybir.dt.float32
bf16 = mybir.dt.bfloat16
AF = mybir.ActivationFunctionType
ALU = mybir.AluOpType

_cache = {}
USE_SILU = True


def _build_program(rep=1):
    nc = bacc.Bacc("TRN2", target_bir_lowering=False, debug=False, num_devices=8)

    xext_d = nc.dram_tensor("xext", [TEXT, C], f32, kind="ExternalInput").ap()
    cos_d = nc.dram_tensor("cosd", [TEXT, 32], f32, kind="ExternalInput").ap()
    sin_d = nc.dram_tensor("sind", [TEXT, 32], f32, kind="ExternalInput").ap()
    mask_d = nc.dram_tensor("maskd", [4 * 128, 640], bf16, kind="ExternalInput").ap()
    s1_d = nc.dram_tensor("s1d", [1, C], f32, kind="ExternalInput").ap()
    s2_d = nc.dram_tensor("s2d", [1, C], f32, kind="ExternalInput").ap()
    qw_d = nc.dram_tensor("qwd", [C, NH * D], bf16, kind="ExternalInput").ap()
    kw_d = nc.dram_tensor("kwd", [C, NKV * D], bf16, kind="ExternalInput").ap()
    vw_d = nc.dram_tensor("vwd", [C, NKV * D], bf16, kind="ExternalInput").ap()
    ow_d = nc.dram_tensor("owd", [NH * D, C], bf16, kind="ExternalInput").ap()
    gw_d = nc.dram_tensor("gwd", [C, HID], bf16, kind="ExternalInput").ap()
    uw_d = nc.dram_tensor("uwd", [C, HID], bf16, kind="ExternalInput").ap()
    dw_d = nc.dram_tensor("dwd", [HID, C], bf16, kind="ExternalInput").ap()

    y_d = nc.dram_tensor("y", [TQ, C], f32, kind="ExternalOutput").ap()
    kurt_d = nc.dram_tensor("kurt", [1, 1], f32, kind="ExternalOutput").ap()

    with tile.TileContext(nc) as tc, ExitStack() as ctx:
      for _rep in range(rep):
        ctx = ExitStack()
        consts = ctx.enter_context(tc.tile_pool(name="consts", bufs=1))
        mid = ctx.enter_context(tc.tile_pool(name="mid", bufs=1))
        scr = ctx.enter_context(tc.tile_pool(name="scr", bufs=2))
        sml = ctx.enter_context(tc.tile_pool(name="sml", bufs=4))

        ident = consts.tile([128, 128], bf16)
        make_identity(nc, ident)
        ones128 = consts.tile([128, 1], f32)
        nc.vector.memset(ones128, 1.0)
        s1b = consts.tile([128, C], f32)
        nc.gpsimd.dma_start(out=s1b, in_=s1_d.to_broadcast([128, C]))
        s2b = consts.tile([128, C], f32)
        nc.gpsimd.dma_start(out=s2b, in_=s2_d.to_broadcast([128, C]))
        epsb = consts.tile([128, 1], f32)
        nc.vector.memset(epsb, 1e-6)
        kurt_cols = mid.tile([128, 4], f32)

        def rmsnorm(x_c, sb, out_bf):
            sq = scr.tile([128, C], f32, tag="scr4k", name="sq")
            ssq = sml.tile([128, 1], f32, tag="ssq", name="ssq")
            nc.scalar.activation(out=sq, in_=x_c, func=AF.Square, accum_out=ssq)
            sd = sml.tile([128, 1], f32, tag="sd", name="sd")
            nc.scalar.activation(out=sd, in_=ssq, func=AF.Sqrt, scale=1.0 / C,
                                 bias=epsb[:])
            rstd = sml.tile([128, 1], f32, tag="rstd", name="rstd")
            nc.vector.reciprocal(out=rstd, in_=sd)
            xnf = scr.tile([128, C], f32, tag="scr4kb", name="xnf")
            nc.vector.tensor_scalar_mul(xnf[:], x_c, rstd[:])
            nc.vector.tensor_mul(out_bf, xnf[:], sb[:])

        def transpose4(tpool, src_bf, dst, dst_chunk0, dst_cols):
            # transpose 4 adjacent [128,128] blocks of src into
            # dst[:, dst_chunk0:dst_chunk0+4, dst_cols]
            tp4 = tpool.tile([128, 512], bf16, tag="tp4", name="tp4")
            for b in range(4):
                nc.tensor.transpose(tp4[:, b * 128:(b + 1) * 128],
                                    src_bf[:, b * 128:(b + 1) * 128], ident[:])
            nc.vector.tensor_copy(
                out=dst[:, dst_chunk0:dst_chunk0 + 4, dst_cols],
                in_=tp4[:].rearrange("p (b n) -> p b n", b=4))

        with tc.tile_pool(name="attn_persist", bufs=1) as ap, \
             tc.tile_pool(name="xpool", bufs=2) as xp, \
             tc.tile_pool(name="ropes", bufs=2) as rp, \
             tc.tile_pool(name="smx", bufs=2) as smx:

            cos_t = ap.tile([128, 8, 32], f32)
            sin_t = ap.tile([128, 8, 32], f32)
            masks_t = ap.tile([128, 4, 640], bf16)
            nc.sync.dma_start(out=cos_t[:], in_=cos_d.rearrange("(rc p) f -> p rc f", p=128))
            nc.sync.dma_start(out=sin_t[:], in_=sin_d.rearrange("(rc p) f -> p rc f", p=128))
            nc.sync.dma_start(out=masks_t[:], in_=mask_d.rearrange("(rc p) f -> p rc f", p=128))
            qw_sb = ap.tile([128, 8, NH * D], bf16)
            kw_sb = ap.tile([128, 8, NKV * D], bf16)
            vw_sb = ap.tile([128, 8, NKV * D], bf16)
            ow_sb = ap.tile([128, 8, C], bf16)
            nc.sync.dma_start(out=qw_sb[:], in_=qw_d.rearrange("(kc p) n -> p kc n", p=128))
            nc.sync.dma_start(out=kw_sb[:], in_=kw_d.rearrange("(kc p) n -> p kc n", p=128))
            nc.sync.dma_start(out=vw_sb[:], in_=vw_d.rearrange("(kc p) n -> p kc n", p=128))
            nc.sync.dma_start(out=ow_sb[:], in_=ow_d.rearrange("(kc p) n -> p kc n", p=128))

            xq_t = ap.tile([128, 4, C], f32)       # query rows of x (residual)
            xnT_t = ap.tile([128, 8, TEXT], bf16)  # rms1(x) transposed [C, t]
            x2_t = mid.tile([128, 4, C], f32)      # x + attn_out
            xn2T_t = mid.tile([128, 8, TQ], bf16)  # rms2(x2) transposed
            KT_t = ap.tile([128, 4, TEXT], bf16)
            V_aug = ap.tile([128, 8, 8, 65], bf16)
            QT_t = ap.tile([128, 8, TQ], bf16)
            OT_pack = ap.tile([128, 8, TQ], bf16)

            def rope(nat_psum_view, rc, out_bf, nheads):
                # nat_psum_view: [128, nheads, 64] psum fp32; out_bf alike bf16
                cos_b = cos_t[:, rc:rc + 1, :].broadcast_to([128, nheads, 32])
                sin_b = sin_t[:, rc:rc + 1, :].broadcast_to([128, nheads, 32])
                lo, hi = nat_psum_view[:, :, 0:32], nat_psum_view[:, :, 32:64]
                t1 = rp.tile([128, nheads, 32], f32, tag="t1", name="t1")
                t2 = rp.tile([128, nheads, 32], f32, tag="t2", name="t2")
                nc.vector.tensor_mul(t1[:], lo, cos_b)
                nc.vector.tensor_mul(t2[:], hi, sin_b)
                nc.vector.tensor_sub(out_bf[:, :, 0:32], t1[:], t2[:])
                t3 = rp.tile([128, nheads, 32], f32, tag="t1", name="t3")
                t4 = rp.tile([128, nheads, 32], f32, tag="t2", name="t4")
                nc.vector.tensor_mul(t3[:], hi, cos_b)
                nc.vector.tensor_mul(t4[:], lo, sin_b)
                nc.vector.tensor_add(out_bf[:, :, 32:64], t3[:], t4[:])

            # ================= phases 1-2: norms + projections ============
            with tc.tile_pool(name="mmA", bufs=3, space="PSUM") as mmA, \
                 tc.tile_pool(name="tppA", bufs=2, space="PSUM") as tppA:

                nc.sync.dma_start(out=xq_t[:],
                                  in_=xext_d[512:].rearrange("(c p) n -> p c n", p=128))
                nc.gpsimd.memset(V_aug[:, :, :, 64:65], 1.0)
                for rc in range(8):
                    if rc >= 4:
                        x_c = xq_t[:, rc - 4, :]
                    else:
                        x_c = xp.tile([128, C], f32, tag="xc", name="xc")
                        nc.sync.dma_start(out=x_c, in_=xext_d[rc * 128:(rc + 1) * 128, :])
                    xnb = scr.tile([128, C], bf16, tag="xnb", name="xnb")
                    rmsnorm(x_c, s1b, xnb[:])
                    for half in range(2):
                        transpose4(tppA, xnb[:, half * 512:(half + 1) * 512],
                                   xnT_t, half * 4, slice(rc * 128, (rc + 1) * 128))

                for rc in range(8):
                    kps = mmA.tile([128, 512], f32, tag="mm", name="kps")
                    for kc in range(8):
                        nc.tensor.matmul(kps[:], xnT_t[:, kc, rc * 128:(rc + 1) * 128],
                                         kw_sb[:, kc, :], start=kc == 0, stop=kc == 7)
                    krb = rp.tile([128, 8, 64], bf16, tag="krb", name="krb")
                    rope(kps[:].rearrange("p (h d) -> p h d", h=8), rc, krb, 8)
                    transpose4(tppA, krb[:].rearrange("p h d -> p (h d)"),
                               KT_t, 0, slice(rc * 128, (rc + 1) * 128))

                    vps = mmA.tile([128, 512], f32, tag="mm", name="vps")
                    for kc in range(8):
                        nc.tensor.matmul(vps[:], xnT_t[:, kc, rc * 128:(rc + 1) * 128],
                                         vw_sb[:, kc, :], start=kc == 0, stop=kc == 7)
                    nc.vector.tensor_copy(out=V_aug[:, rc, :, 0:64],
                                          in_=vps[:].rearrange("p (h d) -> p h d", h=8))

                for icq in range(4):
                    rc = 4 + icq
                    for nh in range(2):
                        qps = mmA.tile([128, 512], f32, tag="mm", name="qps")
                        for kc in range(8):
                            nc.tensor.matmul(qps[:], xnT_t[:, kc, rc * 128:(rc + 1) * 128],
                                             qw_sb[:, kc, nh * 512:(nh + 1) * 512],
                                             start=kc == 0, stop=kc == 7)
                        qrb = rp.tile([128, 8, 64], bf16, tag="krb", name="qrb")
                        rope(qps[:].rearrange("p (h d) -> p h d", h=8), rc, qrb, 8)
                        transpose4(tppA, qrb[:].rearrange("p h d -> p (h d)"),
                                   QT_t, nh * 4, slice(icq * 128, (icq + 1) * 128))

            # ====== phases 3-5: attention + out-proj + rms2, per i-block ==
            with tc.tile_pool(name="spp", bufs=2, space="PSUM") as spp, \
                 tc.tile_pool(name="otp", bufs=1, space="PSUM") as otp, \
                 tc.tile_pool(name="mmB", bufs=2, space="PSUM") as mmB, \
                 tc.tile_pool(name="tppB", bufs=1, space="PSUM") as tppB, \
                 tc.tile_pool(name="dscr", bufs=2, space="DRAM") as dp:

                Ddr = dp.tile([16, TQ], f32, tag="Ddr", name="Ddr")
                for ib in range(4):
                    for rh in range(NH):
                        kv = rh % NKV
                        ko = 64 * (kv % 2)
                        qo = 64 * (rh % 2)
                        ots = otp.tile([65, 128], f32, tag="ot", name="ots")
                        sp_t = spp.tile([128, 640], f32, tag="sp", name="sp_t")
                        for w in range(5):
                            jt = ib + w
                            nc.tensor.matmul(
                                sp_t[:, w * 128:(w + 1) * 128],
                                KT_t[ko:ko + 64, kv // 2, jt * 128:(jt + 1) * 128],
                                QT_t[qo:qo + 64, rh // 2, ib * 128:(ib + 1) * 128],
                                start=True, stop=True)
                        tanh_t = smx.tile([128, 640], f32, tag="tanh", name="tanh_t")
                        nc.scalar.activation(out=tanh_t[:], in_=sp_t[:], func=AF.Tanh)
                        p_bf = smx.tile([128, 640], bf16, tag="pbf", name="p_bf")
                        nc.scalar.activation(out=p_bf[:], in_=tanh_t[:], func=AF.Exp,
                                             scale=SOFT_CAP)
                        nc.vector.tensor_mul(p_bf[:], p_bf[:], masks_t[:, ib, :])
                        for w in range(5):
                            jt = ib + w
                            nc.tensor.matmul(ots[:], V_aug[:, jt, kv, :],
                                             p_bf[:, w * 128:(w + 1) * 128],
                                             start=w == 0, stop=w == 4)
                        nc.vector.tensor_copy(
                            out=OT_pack[qo:qo + 64, rh // 2, ib * 128:(ib + 1) * 128],
                            in_=ots[0:64, :])
                        rinv = sml.tile([1, 128], f32, tag="rinv", name="rinv")
                        nc.vector.reciprocal(out=rinv, in_=ots[64:65, :])
                        nc.sync.dma_start(
                            out=Ddr[rh:rh + 1, ib * 128:(ib + 1) * 128],
                            in_=rinv[:])

                    # normalize OT for this i-block, then out-proj + residual
                    ddr_ap = Ddr[:]
                    for kc in range(8):
                        dbk = smx.tile([128, 128], f32, tag="dbk", name="dbk")
                        gather = bass.AP(tensor=ddr_ap.tensor,
                                         offset=ddr_ap.offset + kc * 2 * TQ + ib * 128,
                                         ap=[[TQ, 2], [0, 64], [1, 128]])
                        nc.sync.dma_start(out=dbk[:], in_=gather)
                        nc.vector.tensor_mul(
                            OT_pack[:, kc, ib * 128:(ib + 1) * 128],
                            OT_pack[:, kc, ib * 128:(ib + 1) * 128], dbk[:])
                    for nh in range(2):
                        ops_ = mmB.tile([128, 512], f32, tag="mm", name="ops_")
                        for kc in range(8):
                            nc.tensor.matmul(ops_[:], OT_pack[:, kc, ib * 128:(ib + 1) * 128],
                                             ow_sb[:, kc, nh * 512:(nh + 1) * 512],
                                             start=kc == 0, stop=kc == 7)
                        nc.vector.tensor_add(x2_t[:, ib, nh * 512:(nh + 1) * 512], ops_[:],
                                             xq_t[:, ib, nh * 512:(nh + 1) * 512])
                    xnb = scr.tile([128, C], bf16, tag="xnb", name="xnb2")
                    rmsnorm(x2_t[:, ib, :], s2b, xnb[:])
                    for half in range(2):
                        transpose4(tppB, xnb[:, half * 512:(half + 1) * 512],
                                   xn2T_t, half * 4, slice(ib * 128, (ib + 1) * 128))

        # ================= phase 6-7: MLP + output =======================
        with tc.tile_pool(name="mlp_persist", bufs=1) as mp, \
             tc.tile_pool(name="wstream", bufs=2) as ws, \
             tc.tile_pool(name="ypool", bufs=1) as yp, \
             tc.tile_pool(name="kstat", bufs=2) as kp, \
             tc.tile_pool(name="mmC", bufs=3, space="PSUM") as mmC:

            hT_t = mp.tile([128, 32, TQ], bf16)
            for nh in range(8):
                gcol = ws.tile([128, 8, 512], bf16, tag="gcol")
                ucol = ws.tile([128, 8, 512], bf16, tag="ucol")
                nc.sync.dma_start(
                    out=gcol[:],
                    in_=gw_d[:, nh * 512:(nh + 1) * 512].rearrange("(kc p) n -> p kc n", p=128))
                nc.sync.dma_start(
                    out=ucol[:],
                    in_=uw_d[:, nh * 512:(nh + 1) * 512].rearrange("(kc p) n -> p kc n", p=128))
                for bp in range(2):
                    gps = mmC.tile([128, 1024], f32, tag="mm", name="gps")
                    ups = mmC.tile([128, 1024], f32, tag="mm", name="ups")
                    for h2 in range(2):
                        b2 = bp * 2 + h2
                        for kc in range(8):
                            nc.tensor.matmul(gps[:, h2 * 512:(h2 + 1) * 512],
                                             gcol[:, kc, b2 * 128:(b2 + 1) * 128],
                                             xn2T_t[:, kc, :], start=kc == 0, stop=kc == 7)
                        for kc in range(8):
                            nc.tensor.matmul(ups[:, h2 * 512:(h2 + 1) * 512],
                                             ucol[:, kc, b2 * 128:(b2 + 1) * 128],
                                             xn2T_t[:, kc, :], start=kc == 0, stop=kc == 7)
                    gs = ws.tile([128, 1024], f32, tag="gs", name="gs")
                    hslc = (hT_t[:, nh * 4 + bp * 2:nh * 4 + bp * 2 + 2, :]
                            .rearrange("p a b -> p (a b)"))
                    if USE_SILU:
                        nc.scalar.activation(out=gs[:], in_=gps[:], func=AF.Silu)
                        nc.vector.tensor_mul(hslc, gs[:], ups[:])
                    else:
                        nc.scalar.activation(out=gs[:], in_=gps[:], func=AF.Sigmoid)
                        gf = ws.tile([128, 1024], f32, tag="hbf", name="gf")
                        nc.vector.tensor_mul(gf[:], gs[:], gps[:])
                        nc.vector.tensor_mul(hslc, gf[:], ups[:])

            y_ts = [yp.tile([128, C], f32, tag=f"yc{mc}", name=f"yc{mc}")
                    for mc in range(4)]
            for nh2 in range(2):
                dwh = ws.tile([128, 32, 512], bf16, tag="dwh", bufs=1, name="dwh")
                nc.sync.dma_start(
                    out=dwh[:],
                    in_=dw_d[:, nh2 * 512:(nh2 + 1) * 512].rearrange("(kc p) n -> p kc n", p=128))
                for mc in range(4):
                    dps = mmC.tile([128, 512], f32, tag="mm", name="dps")
                    for kc in range(32):
                        nc.tensor.matmul(dps[:], hT_t[:, kc, mc * 128:(mc + 1) * 128],
                                         dwh[:, kc, :], start=kc == 0, stop=kc == 31)
                    nc.vector.tensor_add(y_ts[mc][:, nh2 * 512:(nh2 + 1) * 512], dps[:],
                                         x2_t[:, mc, nh2 * 512:(nh2 + 1) * 512])

            for mc in range(4):
                y_c = y_ts[mc]
                nc.sync.dma_start(out=y_d[mc * 128:(mc + 1) * 128, :], in_=y_c[:])
                stats = kp.tile([128, 2, 6], f32, tag="stats", name="stats")
                for sg in range(2):
                    nc.vector.bn_stats(out=stats[:, sg, :], in_=y_c[:, sg * 512:(sg + 1) * 512])
                mv = kp.tile([128, 2], f32, tag="mv", name="mv")
                nc.vector.bn_aggr(out=mv[:], in_=stats[:])
                c_t = scr.tile([128, C], f32, tag="scr4k", name="c_t")
                nc.vector.tensor_scalar_sub(c_t[:], y_c[:], mv[:, 0:1])
                c2 = scr.tile([128, C], f32, tag="scr4kb", name="c2")
                nc.scalar.activation(out=c2[:], in_=c_t[:], func=AF.Square)
                c4sc = scr.tile([128, C], f32, tag="scr4k", name="c4sc")
                c4s = sml.tile([128, 1], f32, tag="c4s", name="c4s")
                nc.scalar.activation(out=c4sc[:], in_=c2[:], func=AF.Square, accum_out=c4s)
                v2 = sml.tile([128, 1], f32, tag="v2", name="v2")
                nc.vector.tensor_mul(v2[:], mv[:, 1:2], mv[:, 1:2])
                v2b = sml.tile([128, 1], f32, tag="v2b", name="v2b")
                nc.vector.tensor_scalar_add(v2b[:], v2[:], 1e-6)
                v2i = sml.tile([128, 1], f32, tag="v2i", name="v2i")
                nc.vector.reciprocal(out=v2i, in_=v2b[:])
                kr = sml.tile([128, 1], f32, tag="kr", name="kr")
                nc.vector.tensor_mul(kr[:], c4s[:], v2i[:])
                kr2 = sml.tile([128, 1], f32, tag="kr2", name="kr2")
                nc.vector.tensor_scalar(out=kr2[:], in0=kr[:], scalar1=1.0 / C,
                                        scalar2=-3.0, op0=ALU.mult, op1=ALU.add)
                nc.vector.tensor_scalar_max(kurt_cols[:, mc:mc + 1], kr2[:], 0.0)

            krow = sml.tile([128, 1], f32, tag="krow", name="krow")
            nc.vector.tensor_reduce(out=krow[:], in_=kurt_cols[:],
                                    axis=mybir.AxisListType.X, op=ALU.add)
            kps_ = mmC.tile([64, 512], f32, tag="mm", name="kps_")
            nc.tensor.matmul(kps_[0:1, 0:1], ones128[:], krow[:], start=True, stop=True)
            kurt_sc = sml.tile([1, 1], f32, tag="ksc", name="ksc")
            nc.vector.tensor_copy(out=kurt_sc[:], in_=kps_[0:1, 0:1])
            nc.sync.dma_start(out=kurt_d, in_=kurt_sc[:])

        ctx.close()
    nc.compile()
    return nc


def _host_inputs(inputs):
    """Returns list of 8 per-core input maps."""
    x = np.asarray(inputs["x"], np.float32)
    rms1 = np.asarray(inputs["rms1_scale"], np.float32)
    rms2 = np.asarray(inputs["rms2_scale"], np.float32)

    def tobf(a):
        return np.ascontiguousarray(a.astype(ml_dtypes.bfloat16))

    qw = tobf(np.asarray(inputs["q_kernel"], np.float32).reshape(C, NH * D) * (0.125 / SOFT_CAP))
    kw = tobf(np.asarray(inputs["k_kernel"], np.float32).reshape(C, NKV * D))
    vw = tobf(np.asarray(inputs["v_kernel"], np.float32).reshape(C, NKV * D))
    ow = tobf(np.asarray(inputs["out_kernel"], np.float32).reshape(NH * D, C))
    gw = tobf(np.asarray(inputs["gate_kernel"], np.float32))
    uw = tobf(np.asarray(inputs["up_kernel"], np.float32))
    dw = tobf(np.asarray(inputs["down_kernel"], np.float32))
    s1 = np.ascontiguousarray((1.0 + rms1).reshape(1, C))
    s2 = np.ascontiguousarray((1.0 + rms2).reshape(1, C))

    # rope tables (fp32, replicating reference formula)
    fraction = (np.arange(0, D, 2, dtype=np.float32) / np.float32(D)).astype(np.float32)
    rot_freq = (np.float32(1.0) / np.power(np.float32(1e6), fraction)).astype(np.float32)
    pos = np.arange(T, dtype=np.float32)
    sinu = pos[:, None] * rot_freq[None, :]
    sin_full = np.sin(sinu).astype(np.float32)
    cos_full = np.cos(sinu).astype(np.float32)

    # band mask per query-block: [ib, p, w*128+i] for jl=(ib+w)*128+p,
    # il=ib*128+i; valid iff il < jl <= il + WINDOW (local coords)
    ibs = np.arange(4)[:, None, None, None]
    ps = np.arange(128)[None, :, None, None]
    ws_ = np.arange(5)[None, None, :, None]
    iis = np.arange(128)[None, None, None, :]
    jl4 = (ibs + ws_) * 128 + ps
    il4 = ibs * 128 + iis
    band = (il4 < jl4) & (jl4 <= il4 + WINDOW)

    in_maps = []
    for core in range(8):
        b, s = core // NCHUNKS, core % NCHUNKS
        i0 = s * TQ
        lo = i0 - WINDOW
        x_ext = np.zeros((TEXT, C), np.float32)
        cos_e = np.zeros((TEXT, 32), np.float32)
        sin_e = np.zeros((TEXT, 32), np.float32)
        src_lo = max(lo, 0)
        dst_lo = src_lo - lo
        x_ext[dst_lo:] = x[b, src_lo:i0 + TQ]
        cos_e[dst_lo:] = cos_full[src_lo:i0 + TQ]
        sin_e[dst_lo:] = sin_full[src_lo:i0 + TQ]
        valid = (lo + jl4 >= 0)
        mask_core = np.ascontiguousarray(
            (band & valid).reshape(4, 128, 640).reshape(4 * 128, 640)
            .astype(ml_dtypes.bfloat16))
        in_maps.append({
            "xext": x_ext, "cosd": cos_e, "sind": sin_e, "maskd": mask_core,
            "s1d": s1, "s2d": s2,
            "qwd": qw, "kwd": kw, "vwd": vw, "owd": ow,
            "gwd": gw, "uwd": uw, "dwd": dw,
        })
    return in_maps


def kernel(**inputs):
    if "nc" not in _cache:
        _cache["nc"] = _build_program()
    nc = _cache["nc"]
    in_maps = _host_inputs(inputs)
    trace = bool(int(os.environ.get("TRNK_TRACE", "0")))
    res = run_bass_kernel_spmd(nc, in_maps, core_ids=list(range(8)), trace=trace)
    _cache["last_result"] = res

    x_out = np.empty((B, T, C), np.float32)
    kurt = np.float32(np.asarray(inputs["kurtosis_sum"], np.float32))
    for core in range(8):
        b, s = core // NCHUNKS, core % NCHUNKS
        x_out[b, s * TQ:(s + 1) * TQ] = res.results[core]["y"]
        kurt = np.float32(kurt + res.results[core]["kurt"][0, 0])
    return (x_out, kurt)
